# revision 6
# baseline (speedup 1.0000x reference)
"""Trainium2 Bass kernel: segmented (expert-parallel) LoRA with dropout.

Computes  out = result + scatter_e( (data_e * keep_e * scale) @ A_e^T @ B_e^T )
where keep = (drop_mask >= 0.05), scale = 2.0 / 0.95, and each of the E=8
adapters owns a contiguous batch segment of 2 batches (4096 tokens).

Sharding: expert-parallel — core e gets adapter e's A/B and its batch segment
(data/drop_mask/result slices), so there are no cross-core collectives.

Per-core dataflow, per 128-token tile (32 tiles per core):
  1. DMA in mask/data/result rows [128, 4096] fp32 (HWDGE).
  2. DVE scalar_tensor_tensor: dropped_bf16 = (mask >= 0.05) * data  (one op).
  3. PE transpose 128x128 blocks of dropped -> PSUM (bf16), ACT-copy to SBUF:
     xT[h, tok] layout needed because the PE contracts over the partition dim.
  4. GEMM1: midT[r, tok] += A_chunk[h,r].T @ xT_chunk[h, tok], 32 chunks
     accumulated in one PSUM bank (A is pre-scaled by `scale`).
  5. GEMM2: out_psum[tok, 512h] = midT[r, tok].T @ B_T[r, 512h] per h-block.
  6. DVE tensor_add: final = out_psum + result -> SBUF, DMA out.

Weights are host-packed into the exact SBUF layouts (tiny: 128 KB each):
  a_pk[p, c*R+j] = A[j, c*128+p] * scale   (bf16)
  b_pk[j, h]     = B[h, j]                 (bf16)
"""

import os
import numpy as np
from contextlib import ExitStack

import ml_dtypes

from concourse import bass, bacc, mybir, tile
from concourse.bass_utils import run_bass_kernel_spmd
from concourse.masks import make_identity

# Pin all HWDGE DMA completions to a single semaphore lane (DMAHW0). HWDGE
# DMAs issued from one engine complete in FIFO order anyway, so merging the
# lanes loses nothing — but it collapses multi-lane DMA waits into one sync
# wait per consumer, keeping every instruction within the ISA's sync-wait
# slot budget (scalar_tensor_tensor has a single slot; most others have two).
import concourse.tile_sem_assignment as _tsa
_tsa.NUM_HWDGE_SEMS = 1

# Problem constants (hardcoded per the self-contained-kernel contract).
E = 8
B, S, H, R = 16, 2048, 4096, 16
SEG = B // E
TOK = SEG * S          # tokens per core = 4096
P = 128                # partitions
P_DROP = 0.05
SCALING = 2.0
SCALE = SCALING / (1.0 - P_DROP)

F32 = mybir.dt.float32
BF16 = mybir.dt.bfloat16
BF16_NP = ml_dtypes.bfloat16

LAST_RESULTS = None    # BassKernelResults of the most recent run (for test.py)


def build_nc(tok=TOK, h=H, r=R, num_devices=E):
    """Build the single-core Bass/Tile program (run SPMD on all cores)."""
    nt = tok // P          # token tiles
    hc = h // P            # 128-wide h chunks
    assert tok % P == 0 and h % 512 == 0
    hb = h // 512          # 512-wide h blocks
    cpb = 512 // P         # 128-chunks per 512-block

    # Bacc (not plain Bass): Bacc.compile() runs generate_event_semaphores,
    # which splits multi-waits to satisfy the 1-sync-wait-per-instruction
    # hardware constraint (walrus rejects the raw Tile output otherwise).
    nc = bacc.Bacc("TRN2", target_bir_lowering=False, debug=False,
                   num_devices=num_devices)

    data = nc.dram_tensor("data", [tok, h], F32, kind="ExternalInput").ap()
    mask = nc.dram_tensor("mask", [tok, h], F32, kind="ExternalInput").ap()
    res = nc.dram_tensor("res", [tok, h], F32, kind="ExternalInput").ap()
    a_pk = nc.dram_tensor("a_pk", [P, hc * r], BF16, kind="ExternalInput").ap()
    b_pk = nc.dram_tensor("b_pk", [r, h], BF16, kind="ExternalInput").ap()
    out = nc.dram_tensor("out", [tok, h], F32, kind="ExternalOutput").ap()

    with ExitStack() as ctx:
        tc = ctx.enter_context(tile.TileContext(nc))
        consts = ctx.enter_context(tc.tile_pool(name="consts", bufs=1))
        loads = ctx.enter_context(tc.tile_pool(name="loads", bufs=2))
        work = ctx.enter_context(tc.tile_pool(name="work", bufs=2))
        outp = ctx.enter_context(tc.tile_pool(name="outp", bufs=2))
        pst = ctx.enter_context(tc.tile_pool(name="pst", bufs=2, space="PSUM"))
        psm = ctx.enter_context(tc.tile_pool(name="psm", bufs=2, space="PSUM"))
        pso = ctx.enter_context(tc.tile_pool(name="pso", bufs=4, space="PSUM"))

        ident = consts.tile([P, P], BF16)
        make_identity(nc, ident)
        a_sb = consts.tile([P, hc * r], BF16)
        nc.sync.dma_start(a_sb, a_pk)
        b_sb = consts.tile([r, h], BF16)
        nc.sync.dma_start(b_sb, b_pk)

        for t in range(nt):
            rows = bass.ts(t, P)
            mask_sb = loads.tile([P, h], F32)
            nc.sync.dma_start(mask_sb, mask[rows, :])
            data_sb = loads.tile([P, h], F32)
            nc.sync.dma_start(data_sb, data[rows, :])
            res_sb = loads.tile([P, h], F32)
            nc.sync.dma_start(res_sb, res[rows, :])

            # dropped = (mask >= p) * data, cast to bf16 (scale folded into A).
            # Two DVE ops rather than one fused scalar_tensor_tensor: the STT
            # ISA struct has a single sync-wait slot, which the scheduler
            # overflows once WAR deps appear.
            keep_sb = work.tile([P, h], BF16)
            nc.vector.tensor_scalar(
                keep_sb, mask_sb, P_DROP, None, op0=mybir.AluOpType.is_ge)
            drop_sb = work.tile([P, h], BF16)
            nc.vector.tensor_tensor(
                drop_sb, data_sb, keep_sb, op=mybir.AluOpType.mult)

            # transpose dropped into xT[h-on-partitions, tok]
            xT_sb = work.tile([P, h], BF16)
            for g in range(hb):
                tp_ps = pst.tile([P, 512], BF16)
                for j in range(cpb):
                    c = g * cpb + j
                    nc.tensor.transpose(
                        tp_ps[:, bass.ts(j, P)], drop_sb[:, bass.ts(c, P)], ident)
                nc.scalar.copy(xT_sb[:, bass.ts(g, 512)], tp_ps)

            # GEMM1: midT[r, tok] = sum_c A_chunk^T @ xT_chunk
            midT_ps = psm.tile([r, P], F32)
            for c in range(hc):
                nc.tensor.matmul(
                    midT_ps, lhsT=a_sb[:, bass.ts(c, r)],
                    rhs=xT_sb[:, bass.ts(c, P)],
                    start=(c == 0), stop=(c == hc - 1))
            midT_sb = work.tile([r, P], BF16)
            nc.scalar.copy(midT_sb, midT_ps)

            # GEMM2 + final add, per 512-wide h block
            out_sb = outp.tile([P, h], F32)
            for g in range(hb):
                o_ps = pso.tile([P, 512], F32)
                nc.tensor.matmul(o_ps, lhsT=midT_sb,
                                 rhs=b_sb[:, bass.ts(g, 512)],
                                 start=True, stop=True)
                nc.vector.tensor_add(out_sb[:, bass.ts(g, 512)], o_ps,
                                     res_sb[:, bass.ts(g, 512)])
            nc.sync.dma_start(out[rows, :], out_sb)
    nc.compile()
    return nc


def pack_weights(lora_a, lora_b, h=H, r=R):
    """Pack A (pre-scaled) and B into the SBUF layouts the kernel expects."""
    e = lora_a.shape[0]
    hc = h // P
    a_sc = (np.asarray(lora_a, np.float32) * SCALE).astype(BF16_NP)   # (E,R,H)
    a_pk = np.ascontiguousarray(
        a_sc.reshape(e, r, hc, P).transpose(0, 3, 2, 1)).reshape(e, P, hc * r)
    b_pk = np.ascontiguousarray(
        np.asarray(lora_b, np.float32).astype(BF16_NP).transpose(0, 2, 1))
    return a_pk, b_pk


def kernel(result, data, drop_mask, lora_a, lora_b, _trace=False):
    global LAST_RESULTS
    result = np.asarray(result, np.float32)
    data = np.asarray(data, np.float32)
    drop_mask = np.asarray(drop_mask, np.float32)

    data_e = data.reshape(E, TOK, H)
    mask_e = drop_mask.reshape(E, TOK, H)
    res_e = result.reshape(E, TOK, H)
    a_pk, b_pk = pack_weights(lora_a, lora_b)

    nc = build_nc()
    in_maps = [
        {"data": data_e[e], "mask": mask_e[e], "res": res_e[e],
         "a_pk": a_pk[e], "b_pk": b_pk[e]}
        for e in range(E)
    ]
    LAST_RESULTS = run_bass_kernel_spmd(
        nc, in_maps, core_ids=list(range(E)), trace=_trace)
    outs = [LAST_RESULTS.results[e]["out"] for e in range(E)]
    return np.stack(outs).reshape(B, S, H)


if __name__ == "__main__":
    # smoke test with random data
    rng = np.random.default_rng(0)
    inputs = {
        "result": rng.standard_normal((B, S, H), dtype=np.float32),
        "data": rng.standard_normal((B, S, H), dtype=np.float32),
        "drop_mask": rng.random((B, S, H), dtype=np.float32),
        "lora_a": (rng.standard_normal((E, R, H), dtype=np.float32) * 0.02),
        "lora_b": (rng.standard_normal((E, H, R), dtype=np.float32) * 0.02),
    }
    out = kernel(**inputs)
    print("out", out.shape, out.dtype)
